# revision 30
# baseline (speedup 1.0000x reference)
"""VQ codebook EMA-update kernel for 8 Trainium2 NeuronCores.

Problem: x (4, 64, 256, 256) f32, means (512, 64) f32.
tokens = x viewed as (4, 64, 65536) -> 262144 tokens of dim 64 (d-major in HBM).
dists = tokens @ means.T; buckets = argmax; bins/sums segment-sums;
means_new = normalize(sums) (keep old where empty); out = 0.999*means + 0.001*means_new.

Sharding: data-parallel over tokens. Core i gets x[i//2, :, (i%2)*32768:...]
-> [64, 32768] contiguous slab. Per-codebook sums+bins are AllReduced across
the 8 cores; every core then computes the (identical) final update.

Per-core design (software-pipelined, chunks of 128 tokens):
  - x and meansT are typed float32r so the dists matmul streams at 1 col/cycle
    (plain fp32 matmul is 4x slower on the PE).
  - dists_psum[128, 2, 512] = two chunks' matmuls (lhsT=x_chunk[64,128] is the
    stationary operand, rhs=meansT[64,512] streams); one batched DVE
    reduce_max(negate) over both chunks amortizes the op overhead.
  - onehot-1 in {-1(loser), 0(winner)} via ACT: Sign(dists - max) with the
    per-token -max as the activation bias; output bf16.  Using {-1,0} instead
    of {0,1} lets ACT produce the scatter rhs in ONE pass; the constant shift
    is removed after the AllReduce using sum_c(sums - rowtot) = -511*rowtot.
  - token-major xaug[128, 66] (x^T | ones) built by PE transpose of the
    x chunk (+ones row) and one batched DVE copy per 4 chunks (bf16).
  - scatter: acc_psum[66, 512] += matmul(lhsT=xaug, rhs=onehot-1), PSUM
    accumulation across all 256 chunks (bf16 inputs, fp32 accumulate).
  - acc -> DRAM, AllReduce(add) over 8 cores, rowtotal recovery, then
    normalize / empty-bucket keep-old / EMA blend, DMA out (all fp32).
"""

import sys

sys.path.insert(0, "/opt/trn_rl_repo")

import numpy as np

import concourse.bacc as bacc
import concourse.bass as bass
import concourse.mybir as mybir
import concourse.tile as tile
from concourse.bass_utils import run_bass_kernel_spmd
from concourse.masks import make_identity

N_CORES = 8
B, CH, H, W = 4, 64, 256, 256
L_TOT = B * H * W            # 262144 tokens
L = L_TOT // N_CORES         # 32768 tokens per core
D = 64                       # token dim
C = 512                      # codebook size
TOK = 128                    # tokens per matmul chunk
TOKD = 512                   # tokens per DMA tile
N_CHUNK = L // TOK           # 256
DECAY = 0.999
EPS = 1e-12

FP32 = mybir.dt.float32
FP32R = mybir.dt.float32r
BF16 = mybir.dt.bfloat16


def build_kernel(n_chunks: int = N_CHUNK) -> bass.Bass:
    nc = bacc.Bacc(None, target_bir_lowering=False, debug=False)

    x_ext = nc.dram_tensor("x", [D, L], FP32R, kind="ExternalInput")
    means_ext = nc.dram_tensor("means", [C, D], FP32, kind="ExternalInput")
    out_ext = nc.dram_tensor("out", [C, D], FP32, kind="ExternalOutput")

    cc_in = nc.dram_tensor("cc_in", [D + 1, C], FP32)
    cc_out = nc.dram_tensor("cc_out", [D + 1, C], FP32, addr_space="Shared")

    with tile.TileContext(nc) as tc:
        with (
            tc.tile_pool(name="const", bufs=1) as const_pool,
            tc.tile_pool(name="xc", bufs=4) as xc_pool,
            tc.tile_pool(name="oh", bufs=10) as oh_pool,
            tc.tile_pool(name="small", bufs=8) as small_pool,
            tc.tile_pool(name="fin", bufs=2) as fin_pool,
            tc.tile_pool(name="dists", bufs=2, space="PSUM") as dists_pool,
            tc.tile_pool(name="tp", bufs=2, space="PSUM") as tp_pool,
            tc.tile_pool(name="acc", bufs=1, space="PSUM") as acc_pool,
        ):
            ident = const_pool.tile([128, 128], FP32)
            make_identity(nc, ident[:])

            # Load means [512, 64] as 4 tiles of [128, 64]; build meansT [64, 512].
            m_sb = []
            meansT = const_pool.tile([D, C], FP32R)
            for g in range(4):
                mg = const_pool.tile([128, D], FP32, tag=f"m{g}")
                nc.sync.dma_start(out=mg[:], in_=means_ext[g * 128:(g + 1) * 128, :])
                m_sb.append(mg)
                mt_ps = tp_pool.tile([D, 128], FP32, tag="tp")
                nc.tensor.transpose(out=mt_ps[:], in_=mg[:], identity=ident[:])
                nc.vector.tensor_copy(out=meansT[:, g * 128:(g + 1) * 128], in_=mt_ps[:])

            # Persistent PSUM accumulator: the Sign-shifted scatter sums.T,
            # acc[m, c] = sums_aug.T[m, c] - rowtotal[m].  Since each token
            # lands in exactly one bucket, sum_c sums_aug[m, c] = rowtotal[m],
            # so sum_c acc[m, :] = -511 * rowtotal[m]; rowtotal is recovered
            # at the end with one reduce instead of a per-chunk matmul.
            acc = acc_pool.tile([D + 2, C], FP32)

            n_dma = L // TOKD
            subs = TOKD // TOK          # 4 chunks per DMA tile
            RG = 2                      # chunks whose dists share one reduce
            LAG = 8                     # pipeline depth (chunks) before scatter

            pending = {}  # ci -> (xaug4, slot, ohm1)
            state = {}

            def produce(ci):
                i, s = divmod(ci, subs)
                if s == 0:
                    xc = xc_pool.tile([D + 1, TOKD], FP32R, tag="xc")
                    nc.sync.dma_start(out=xc[:D, :],
                                      in_=x_ext[:, i * TOKD:(i + 1) * TOKD])
                    nc.gpsimd.memset(xc[D:D + 1, :].bitcast(FP32), 1.0)
                    state["xc"] = xc
                    state["tp"] = tp_pool.tile([TOK, subs, D + 2], FP32, tag="tp", name="tp4")
                xc = state["xc"]
                xsub = xc[0:D, s * TOK:(s + 1) * TOK]
                xsub_aug = xc[:, s * TOK:(s + 1) * TOK]

                g = ci % RG
                if g == 0:
                    state["dists"] = dists_pool.tile([TOK, RG, C], FP32, tag="dists", name="dists2")
                dists = state["dists"]
                nc.tensor.matmul(out=dists[:, g, :], lhsT=xsub,
                                 rhs=meansT[:], start=True, stop=True)

                nc.tensor.transpose(out=state["tp"][:, s, 0:D + 1],
                                    in_=xsub_aug.bitcast(FP32),
                                    identity=ident[:D + 1, :D + 1])

                if g == RG - 1:
                    negmax = small_pool.tile([TOK, RG], FP32, tag="negmax")
                    nc.vector.tensor_reduce(out=negmax[:], in_=dists[:],
                                            axis=mybir.AxisListType.X,
                                            op=mybir.AluOpType.max, negate=True)
                    state["negmax"] = negmax
                    # onehot - 1: winner -> 0, losers -> -1  (Sign(d - max))
                    for gg in range(RG):
                        ohm1 = oh_pool.tile([TOK, C], BF16, tag="oh")
                        nc.scalar.activation(
                            out=ohm1[:], in_=dists[:, gg, :],
                            func=mybir.ActivationFunctionType.Sign,
                            bias=negmax[:, gg:gg + 1], scale=1.0)
                        pending[ci - (RG - 1) + gg] = [None, None, ohm1]

                if s == subs - 1:
                    xaug4 = small_pool.tile([TOK, subs, D + 2], BF16, tag="xaug")
                    nc.vector.tensor_copy(out=xaug4[:], in_=state["tp"][:])
                    for ss in range(subs):
                        pending[ci - (subs - 1) + ss][0] = xaug4
                        pending[ci - (subs - 1) + ss][1] = ss

            def consume(ci):
                xaug4, slot, ohm1 = pending.pop(ci)
                nc.tensor.matmul(out=acc[:], lhsT=xaug4[:, slot, :], rhs=ohm1[:],
                                 start=(ci == 0), stop=(ci == n_chunks - 1),
                                 skip_group_check=True)

            for ci in range(n_chunks + LAG):
                if ci < n_chunks:
                    produce(ci)
                if ci >= LAG:
                    consume(ci - LAG)

            # Evacuate accumulators and AllReduce across the 8 cores.
            acc_sb = fin_pool.tile([D + 1, C], FP32, tag="acc_sb")
            nc.vector.tensor_copy(out=acc_sb[:], in_=acc[0:D + 1, :])
            nc.sync.dma_start(out=cc_in[:], in_=acc_sb[:])
            nc.gpsimd.collective_compute(
                "AllReduce",
                mybir.AluOpType.add,
                replica_groups=[list(range(N_CORES))],
                ins=[cc_in[:]],
                outs=[cc_out[:]],
            )
            red_raw = fin_pool.tile([D + 1, C], FP32, tag="red_raw")
            nc.sync.dma_start(out=red_raw[:], in_=cc_out[:])
            # rowtotal = -sum_c(red_raw) / 511; sums_aug = red_raw + rowtotal
            rowt = fin_pool.tile([D + 1, 1], FP32, tag="rowt")
            nc.vector.reduce_sum(out=rowt[:], in_=red_raw[:],
                                 axis=mybir.AxisListType.X)
            nc.vector.tensor_scalar_mul(out=rowt[:], in0=rowt[:],
                                        scalar1=-1.0 / 511.0)
            red = fin_pool.tile([D + 1, C], FP32, tag="red")
            nc.vector.tensor_scalar(out=red[:], in0=red_raw[:],
                                    scalar1=rowt[:], scalar2=None,
                                    op0=mybir.AluOpType.add)

            # Final per-centroid math, 4 groups of 128 centroids.
            for g in range(4):
                sg_ps = tp_pool.tile([128, D + 1], FP32, tag="tp")
                nc.tensor.transpose(out=sg_ps[:], in_=red[:, g * 128:(g + 1) * 128],
                                    identity=ident[:D + 1, :D + 1])
                sg = fin_pool.tile([128, D + 1], FP32, tag="sg")
                nc.vector.tensor_copy(out=sg[:], in_=sg_ps[:])
                sums = sg[:, 0:D]
                bins = sg[:, D:D + 1]

                sq = fin_pool.tile([128, D], FP32, tag="sq")
                nc.vector.tensor_tensor(out=sq[:], in0=sums, in1=sums,
                                        op=mybir.AluOpType.mult)
                nsq = fin_pool.tile([128, 1], FP32, tag="nsq")
                nc.vector.reduce_sum(out=nsq[:], in_=sq[:],
                                     axis=mybir.AxisListType.X)
                norm = fin_pool.tile([128, 1], FP32, tag="norm")
                nc.scalar.activation(out=norm[:], in_=nsq[:],
                                     func=mybir.ActivationFunctionType.Sqrt)
                nc.vector.tensor_scalar_max(out=norm[:], in0=norm[:], scalar1=EPS)
                rnorm = fin_pool.tile([128, 1], FP32, tag="rnorm")
                nc.vector.reciprocal(out=rnorm[:], in_=norm[:])

                mnew = fin_pool.tile([128, D], FP32, tag="mnew")
                nc.vector.tensor_scalar_mul(out=mnew[:], in0=sums, scalar1=rnorm[:])

                # u = DECAY*means + (1-DECAY)*mnew
                u = fin_pool.tile([128, D], FP32, tag="u")
                nc.vector.tensor_scalar_mul(out=u[:], in0=m_sb[g][:], scalar1=DECAY)
                t2 = fin_pool.tile([128, D], FP32, tag="t2")
                nc.vector.tensor_scalar_mul(out=t2[:], in0=mnew[:],
                                            scalar1=1.0 - DECAY)
                nc.vector.tensor_add(out=u[:], in0=u[:], in1=t2[:])

                # where bins == 0 keep the old means
                mask = fin_pool.tile([128, 1], FP32, tag="mask")
                nc.vector.tensor_scalar(out=mask[:], in0=bins, scalar1=0.5,
                                        scalar2=None, op0=mybir.AluOpType.is_lt)
                diff = fin_pool.tile([128, D], FP32, tag="diff")
                nc.vector.tensor_tensor(out=diff[:], in0=m_sb[g][:], in1=u[:],
                                        op=mybir.AluOpType.subtract)
                nc.vector.tensor_scalar_mul(out=diff[:], in0=diff[:], scalar1=mask[:])
                nc.vector.tensor_add(out=u[:], in0=u[:], in1=diff[:])

                nc.sync.dma_start(out=out_ext[g * 128:(g + 1) * 128, :], in_=u[:])

    nc.compile()
    return nc


_NC_CACHE = None


def _get_nc():
    global _NC_CACHE
    if _NC_CACHE is None:
        _NC_CACHE = build_kernel()
    return _NC_CACHE


def _shard_inputs(x: np.ndarray, means: np.ndarray) -> list[dict]:
    x = np.asarray(x, dtype=np.float32).reshape(B, CH, H * W)
    means = np.ascontiguousarray(np.asarray(means, dtype=np.float32))
    in_maps = []
    for i in range(N_CORES):
        b = i // 2
        half = i % 2
        xs = np.ascontiguousarray(x[b, :, half * L:(half + 1) * L])
        in_maps.append({"x": xs, "means": means})
    return in_maps


def kernel(x, means, _trace=False):
    nc = _get_nc()
    in_maps = _shard_inputs(x, means)
    res = run_bass_kernel_spmd(nc, in_maps, list(range(N_CORES)), trace=_trace)
    out = np.asarray(res.results[0]["out"], dtype=np.float32)
    if _trace:
        return out, res
    return out


# revision 33
# speedup vs baseline: 2.4318x; 2.4318x over previous
"""VQ codebook EMA-update kernel for 8 Trainium2 NeuronCores.

Problem: x (4, 64, 256, 256) f32, means (512, 64) f32.
tokens = x viewed as (4, 64, 65536) -> 262144 tokens of dim 64 (d-major in HBM).
dists = tokens @ means.T; buckets = argmax; bins/sums segment-sums;
means_new = normalize(sums) (keep old where empty); out = 0.999*means + 0.001*means_new.

Sharding: data-parallel over tokens. Core i gets x[i//2, :, (i%2)*32768:...]
-> [64, 32768] contiguous slab. Per-codebook sums+bins are AllReduced across
the 8 cores; every core then computes the (identical) final update.

Per-core design (software-pipelined, chunks of 128 tokens):
  - x and meansT are typed float32r so the dists matmul streams at 1 col/cycle
    (plain fp32 matmul is 4x slower on the PE).
  - dists_psum[128, 3, 512] = three chunks' matmuls (lhsT=x_chunk[64,128] is
    the stationary operand, rhs=meansT[64,512] streams); one batched DVE
    reduce_max(negate) over the group amortizes the op overhead.
  - onehot-1 in {-1(loser), 0(winner)} via ACT: Sign(dists - max) with the
    per-token -max as the activation bias; output bf16.  Using {-1,0} instead
    of {0,1} lets ACT produce the scatter rhs in ONE pass; the constant shift
    is removed after the AllReduce using sum_c(sums - rowtot) = -511*rowtot.
  - token-major xaug[128, 66] (x^T | ones) built by PE transpose of the
    x chunk (+ones row) and one batched DVE copy per 4 chunks (bf16).
  - scatter: acc_psum[66, 512] += matmul(lhsT=xaug, rhs=onehot-1), PSUM
    accumulation across all 256 chunks (bf16 inputs, fp32 accumulate).
  - acc -> DRAM, AllReduce(add) over 8 cores, rowtotal recovery, then
    normalize / empty-bucket keep-old / EMA blend, DMA out (all fp32).
"""

import sys

sys.path.insert(0, "/opt/trn_rl_repo")

import numpy as np

import concourse.bacc as bacc
import concourse.bass as bass
import concourse.mybir as mybir
import concourse.tile as tile
from concourse.bass_utils import run_bass_kernel_spmd
from concourse.masks import make_identity

N_CORES = 8
B, CH, H, W = 4, 64, 256, 256
L_TOT = B * H * W            # 262144 tokens
L = L_TOT // N_CORES         # 32768 tokens per core
D = 64                       # token dim
C = 512                      # codebook size
TOK = 128                    # tokens per matmul chunk
TOKD = 512                   # tokens per DMA tile
N_CHUNK = L // TOK           # 256
DECAY = 0.999
EPS = 1e-12

FP32 = mybir.dt.float32
FP32R = mybir.dt.float32r
BF16 = mybir.dt.bfloat16


def build_kernel(n_chunks: int = N_CHUNK) -> bass.Bass:
    nc = bacc.Bacc(None, target_bir_lowering=False, debug=False)

    x_ext = nc.dram_tensor("x", [D, L], FP32R, kind="ExternalInput")
    means_ext = nc.dram_tensor("means", [C, D], FP32, kind="ExternalInput")
    out_ext = nc.dram_tensor("out", [C, D], FP32, kind="ExternalOutput")

    cc_in = nc.dram_tensor("cc_in", [D + 1, C], FP32)
    cc_out = nc.dram_tensor("cc_out", [D + 1, C], FP32, addr_space="Shared")

    with tile.TileContext(nc) as tc:
        with (
            tc.tile_pool(name="const", bufs=1) as const_pool,
            tc.tile_pool(name="xc", bufs=4) as xc_pool,
            tc.tile_pool(name="oh", bufs=10) as oh_pool,
            tc.tile_pool(name="small", bufs=8) as small_pool,
            tc.tile_pool(name="fin", bufs=2) as fin_pool,
            tc.tile_pool(name="dists", bufs=2, space="PSUM") as dists_pool,
            tc.tile_pool(name="tp", bufs=1, space="PSUM") as tp_pool,
            tc.tile_pool(name="acc", bufs=1, space="PSUM") as acc_pool,
        ):
            ident = const_pool.tile([128, 128], FP32)
            make_identity(nc, ident[:])

            # Load means [512, 64] as 4 tiles of [128, 64]; build meansT [64, 512].
            m_sb = []
            meansT = const_pool.tile([D, C], FP32R)
            for g in range(4):
                mg = const_pool.tile([128, D], FP32, tag=f"m{g}")
                nc.sync.dma_start(out=mg[:], in_=means_ext[g * 128:(g + 1) * 128, :])
                m_sb.append(mg)
                mt_ps = tp_pool.tile([D, 128], FP32, tag="tp")
                nc.tensor.transpose(out=mt_ps[:], in_=mg[:], identity=ident[:])
                nc.vector.tensor_copy(out=meansT[:, g * 128:(g + 1) * 128], in_=mt_ps[:])

            # Persistent PSUM accumulator: the Sign-shifted scatter sums.T,
            # acc[m, c] = sums_aug.T[m, c] - rowtotal[m].  Since each token
            # lands in exactly one bucket, sum_c sums_aug[m, c] = rowtotal[m],
            # so sum_c acc[m, :] = -511 * rowtotal[m]; rowtotal is recovered
            # at the end with one reduce instead of a per-chunk matmul.
            acc = acc_pool.tile([D + 2, C], FP32)

            n_dma = L // TOKD
            subs = TOKD // TOK          # 4 chunks per DMA tile
            RG = 3                      # chunks whose dists share one reduce
            LAG = 8                     # pipeline depth (chunks) before scatter

            pending = {}  # ci -> (xaug4, slot, ohm1)
            state = {}

            def produce(ci):
                pending[ci] = [None, None, None]
                i, s = divmod(ci, subs)
                if s == 0:
                    xc = xc_pool.tile([D + 1, TOKD], FP32R, tag="xc")
                    nc.sync.dma_start(out=xc[:D, :],
                                      in_=x_ext[:, i * TOKD:(i + 1) * TOKD])
                    nc.gpsimd.memset(xc[D:D + 1, :].bitcast(FP32), 1.0)
                    state["xc"] = xc
                    state["tp"] = tp_pool.tile([TOK, subs, D + 2], FP32, tag="tp", name="tp4")
                xc = state["xc"]
                xsub = xc[0:D, s * TOK:(s + 1) * TOK]
                xsub_aug = xc[:, s * TOK:(s + 1) * TOK]

                g = ci % RG
                if g == 0:
                    state["dists"] = dists_pool.tile([TOK, RG, C], FP32, tag="dists", name="dists2")
                dists = state["dists"]
                nc.tensor.matmul(out=dists[:, g, :], lhsT=xsub,
                                 rhs=meansT[:], start=True, stop=True)

                nc.tensor.transpose(out=state["tp"][:, s, 0:D + 1],
                                    in_=xsub_aug.bitcast(FP32),
                                    identity=ident[:D + 1, :D + 1])

                if g == RG - 1 or ci == n_chunks - 1:
                    ng = g + 1  # chunks in this (possibly partial) group
                    negmax = small_pool.tile([TOK, RG], FP32, tag="negmax")
                    nc.vector.tensor_reduce(out=negmax[:, 0:ng],
                                            in_=dists[:, 0:ng, :],
                                            axis=mybir.AxisListType.X,
                                            op=mybir.AluOpType.max, negate=True)
                    # onehot - 1: winner -> 0, losers -> -1  (Sign(d - max))
                    for gg in range(ng):
                        ohm1 = oh_pool.tile([TOK, C], BF16, tag="oh")
                        nc.scalar.activation(
                            out=ohm1[:], in_=dists[:, gg, :],
                            func=mybir.ActivationFunctionType.Sign,
                            bias=negmax[:, gg:gg + 1], scale=1.0)
                        pending[ci - g + gg][2] = ohm1

                if s == subs - 1:
                    xaug4 = small_pool.tile([TOK, subs, D + 2], BF16, tag="xaug")
                    nc.vector.tensor_copy(out=xaug4[:], in_=state["tp"][:])
                    for ss in range(subs):
                        pending[ci - (subs - 1) + ss][0] = xaug4
                        pending[ci - (subs - 1) + ss][1] = ss

            def consume(ci):
                xaug4, slot, ohm1 = pending.pop(ci)
                nc.tensor.matmul(out=acc[:], lhsT=xaug4[:, slot, :], rhs=ohm1[:],
                                 start=(ci == 0), stop=(ci == n_chunks - 1),
                                 skip_group_check=True)

            for ci in range(n_chunks + LAG):
                if ci < n_chunks:
                    produce(ci)
                if ci >= LAG:
                    consume(ci - LAG)

            # Evacuate accumulators and AllReduce across the 8 cores.
            acc_sb = fin_pool.tile([D + 1, C], FP32, tag="acc_sb")
            nc.vector.tensor_copy(out=acc_sb[:], in_=acc[0:D + 1, :])
            nc.sync.dma_start(out=cc_in[:], in_=acc_sb[:])
            nc.gpsimd.collective_compute(
                "AllReduce",
                mybir.AluOpType.add,
                replica_groups=[list(range(N_CORES))],
                ins=[cc_in[:]],
                outs=[cc_out[:]],
            )
            red_raw = fin_pool.tile([D + 1, C], FP32, tag="red_raw")
            nc.sync.dma_start(out=red_raw[:], in_=cc_out[:])
            # rowtotal = -sum_c(red_raw) / 511; sums_aug = red_raw + rowtotal
            rowt = fin_pool.tile([D + 1, 1], FP32, tag="rowt")
            nc.vector.reduce_sum(out=rowt[:], in_=red_raw[:],
                                 axis=mybir.AxisListType.X)
            nc.vector.tensor_scalar_mul(out=rowt[:], in0=rowt[:],
                                        scalar1=-1.0 / 511.0)
            red = fin_pool.tile([D + 1, C], FP32, tag="red")
            nc.vector.tensor_scalar(out=red[:], in0=red_raw[:],
                                    scalar1=rowt[:], scalar2=None,
                                    op0=mybir.AluOpType.add)

            # Final per-centroid math, 4 groups of 128 centroids.
            for g in range(4):
                sg_ps = tp_pool.tile([128, D + 1], FP32, tag="tp")
                nc.tensor.transpose(out=sg_ps[:], in_=red[:, g * 128:(g + 1) * 128],
                                    identity=ident[:D + 1, :D + 1])
                sg = fin_pool.tile([128, D + 1], FP32, tag="sg")
                nc.vector.tensor_copy(out=sg[:], in_=sg_ps[:])
                sums = sg[:, 0:D]
                bins = sg[:, D:D + 1]

                sq = fin_pool.tile([128, D], FP32, tag="sq")
                nc.vector.tensor_tensor(out=sq[:], in0=sums, in1=sums,
                                        op=mybir.AluOpType.mult)
                nsq = fin_pool.tile([128, 1], FP32, tag="nsq")
                nc.vector.reduce_sum(out=nsq[:], in_=sq[:],
                                     axis=mybir.AxisListType.X)
                norm = fin_pool.tile([128, 1], FP32, tag="norm")
                nc.scalar.activation(out=norm[:], in_=nsq[:],
                                     func=mybir.ActivationFunctionType.Sqrt)
                nc.vector.tensor_scalar_max(out=norm[:], in0=norm[:], scalar1=EPS)
                rnorm = fin_pool.tile([128, 1], FP32, tag="rnorm")
                nc.vector.reciprocal(out=rnorm[:], in_=norm[:])

                mnew = fin_pool.tile([128, D], FP32, tag="mnew")
                nc.vector.tensor_scalar_mul(out=mnew[:], in0=sums, scalar1=rnorm[:])

                # u = DECAY*means + (1-DECAY)*mnew
                u = fin_pool.tile([128, D], FP32, tag="u")
                nc.vector.tensor_scalar_mul(out=u[:], in0=m_sb[g][:], scalar1=DECAY)
                t2 = fin_pool.tile([128, D], FP32, tag="t2")
                nc.vector.tensor_scalar_mul(out=t2[:], in0=mnew[:],
                                            scalar1=1.0 - DECAY)
                nc.vector.tensor_add(out=u[:], in0=u[:], in1=t2[:])

                # where bins == 0 keep the old means
                mask = fin_pool.tile([128, 1], FP32, tag="mask")
                nc.vector.tensor_scalar(out=mask[:], in0=bins, scalar1=0.5,
                                        scalar2=None, op0=mybir.AluOpType.is_lt)
                diff = fin_pool.tile([128, D], FP32, tag="diff")
                nc.vector.tensor_tensor(out=diff[:], in0=m_sb[g][:], in1=u[:],
                                        op=mybir.AluOpType.subtract)
                nc.vector.tensor_scalar_mul(out=diff[:], in0=diff[:], scalar1=mask[:])
                nc.vector.tensor_add(out=u[:], in0=u[:], in1=diff[:])

                nc.sync.dma_start(out=out_ext[g * 128:(g + 1) * 128, :], in_=u[:])

    nc.compile()
    return nc


_NC_CACHE = None


def _get_nc():
    global _NC_CACHE
    if _NC_CACHE is None:
        _NC_CACHE = build_kernel()
    return _NC_CACHE


def _shard_inputs(x: np.ndarray, means: np.ndarray) -> list[dict]:
    x = np.asarray(x, dtype=np.float32).reshape(B, CH, H * W)
    means = np.ascontiguousarray(np.asarray(means, dtype=np.float32))
    in_maps = []
    for i in range(N_CORES):
        b = i // 2
        half = i % 2
        xs = np.ascontiguousarray(x[b, :, half * L:(half + 1) * L])
        in_maps.append({"x": xs, "means": means})
    return in_maps


def kernel(x, means, _trace=False):
    nc = _get_nc()
    in_maps = _shard_inputs(x, means)
    res = run_bass_kernel_spmd(nc, in_maps, list(range(N_CORES)), trace=_trace)
    out = np.asarray(res.results[0]["out"], dtype=np.float32)
    if _trace:
        return out, res
    return out
